# revision 1
# baseline (speedup 1.0000x reference)
"""Trainium2 Bass kernel v2 for nn_Encoder_17918603559377 (4-layer sparse-attention
encoder, top-16 per row, B=2 S=1024 D=512 H=8).

Sharding: 8 cores; core c handles batch c//4, heads {2r, 2r+1} where r = c%4.
Per layer the partial y = o @ Wo are AllReduce-summed (bf16, split in two
token-halves so the first AR overlaps the second half of attention compute);
residual + LayerNorm runs redundantly on every core.

Matmul dtypes: projections / scores / y in float32r (1 cyc/row, operands
rounded to ~10 mantissa bits internally), attention probabilities + V path in
fp16. Top-16 selection runs on fp16 exp values (max8 + match_replace + max8),
softmax Z = sum of the 16 kept values, biases bq via rank-1 matmul, bv/bo
folded into the LayerNorm input.
"""

import sys

sys.path.insert(0, "/opt/trn_rl_repo")

import numpy as np

L, B, S, D, H, DK = 4, 2, 1024, 512, 8, 64
TOPK = 16
EPS = 1e-6
SCALE = 1.0 / np.sqrt(DK)
NT = S // 128  # token tiles
NDT = D // 128  # d-dim tiles

_COMPILED = None


def _register_dve_ops():
    """Two custom DVE ops:
    SELSC_GE_ANT: p = select(e >= e16, e * invZ, 0)   (s0=e16, s1=invZ)
    SELZ_GE_ANT:  e2 = select(e >= a8, 0, e)          (s0=a8; drops top-8)
    """
    from concourse.dve_ops import DveOp, OPS
    import concourse.dve_ops as dops
    from concourse.dve_spec import Spec, Src0, C0, C1, Zero, select, lower
    from concourse.dve_uop import DveOpSpec

    specs = {
        "SELSC_GE_ANT": Spec(
            body=select(Src0 >= C0, Src0 * C1, Zero),
            reference=lambda in0, in1, s0, s1, imm2: np.where(
                in0 >= s0, in0 * s1, 0.0
            ),
        ),
        "SELZ_GE_ANT": Spec(
            body=select(Src0 >= C0, Zero, Src0),
            reference=lambda in0, in1, s0, s1, imm2: np.where(in0 >= s0, 0.0, in0),
        ),
        "SELGE_ANT": Spec(
            body=select(Src0 >= C0, Src0, Zero),
            reference=lambda in0, in1, s0, s1, imm2: np.where(in0 >= s0, in0, 0.0),
        ),
    }
    out = {}
    for name, spec in specs.items():
        existing = next((op for op in OPS if op.name == name), None)
        if existing is not None:
            out[name] = existing
            continue
        op = DveOp(name, spec, subdim=False, uops_sha={})
        OPS.append(op)
        dops._SUB_OPCODE_FOR_NAME[op.name] = dops._CUSTOM_DVE_ROW_BASE + len(OPS) - 1
        for ver in ("v3", "v4"):
            tmp = DveOpSpec(
                name=op.name,
                opcode=dops.get_dve_sub_opcode(op.name),
                uops=lower(spec, ver=ver),
                rd1_en=False,
            )
            op.uops_sha[ver] = tmp.sha(ver)
        out[name] = op
    return out


def _build(sim=False):
    import concourse.bacc as bacc
    import concourse.mybir as mybir
    import concourse.tile as tile
    from concourse import masks

    ops = _register_dve_ops()
    SELZ = ops["SELZ_GE_ANT"]
    SELGE = ops["SELGE_ANT"]
    f32 = mybir.dt.float32
    f32r = mybir.dt.float32r
    f16 = mybir.dt.float16
    bf16 = mybir.dt.bfloat16
    AL = mybir.AluOpType
    AF = mybir.ActivationFunctionType

    nc = bacc.Bacc(
        "TRN2", target_bir_lowering=False, debug=False,
        num_devices=(1 if sim else 8),
    )

    x_d = nc.dram_tensor("x", (S, D), f32, kind="ExternalInput")
    wq_d = nc.dram_tensor("wq", (L, D, 128), f32r, kind="ExternalInput")
    wk_d = nc.dram_tensor("wk", (L, D, 128), f32r, kind="ExternalInput")
    wv_d = nc.dram_tensor("wv", (L, D, 128), f32r, kind="ExternalInput")
    wo_d = nc.dram_tensor("wo", (L, 128, D), f32r, kind="ExternalInput")
    bq_d = nc.dram_tensor("bq", (L, 128), f32r, kind="ExternalInput")
    onesr_d = nc.dram_tensor("onesr", (1, 512), f32r, kind="ExternalInput")
    rows_d = nc.dram_tensor("rows", (3 * L, D), f32, kind="ExternalInput")
    # rows: [0:L] beta, [L:2L] gamma, [2L:3L] B[l] = bv[l] @ Wo[l] + bo[l]
    out_d = nc.dram_tensor("out", (S, D), f32, kind="ExternalOutput")

    HALF = S // 2
    cc_in = [
        [nc.dram_tensor(f"cc_in{l}_{hf}", (HALF, D), bf16, kind="Internal")
         for hf in range(2)] for l in range(L)
    ]
    cc_out = [
        [nc.dram_tensor(f"cc_out{l}_{hf}", (HALF, D), bf16, kind="Internal")
         for hf in range(2)] for l in range(L)
    ]
    GROUPS = [[0, 1, 2, 3], [4, 5, 6, 7]]

    with tile.TileContext(nc) as tc:
        with (
            tc.tile_pool(name="w", bufs=1) as wp,
            tc.tile_pool(name="state", bufs=1) as st,
            tc.tile_pool(name="sb", bufs=3) as sb,
            tc.tile_pool(name="sm", bufs=3) as sm,
            tc.tile_pool(name="ps_w", bufs=2, space="PSUM") as ps_w,
            tc.tile_pool(name="ps_s", bufs=2, space="PSUM") as ps_s,
            tc.tile_pool(name="ps_p", bufs=2, space="PSUM") as ps_p,
            tc.tile_pool(name="ps_o", bufs=2, space="PSUM") as ps_o,
        ):
            ident = wp.tile([128, 128], f32, tag="ident", name="ident")
            masks.make_identity(nc, ident[:])
            ident16 = wp.tile([128, 128], f16, tag="ident16", name="ident16")
            nc.scalar.copy(ident16[:], ident[:])

            # --- weight preload (all layers) ---
            wq_sb = wp.tile([128, L, NDT, 128], f32r, tag="wq", name="wq_sb")
            wk_sb = wp.tile([128, L, NDT, 128], f32r, tag="wk", name="wk_sb")
            wv_sb = wp.tile([128, L, NDT, 128], f32r, tag="wv", name="wv_sb")
            wo_sb = wp.tile([128, L, D], f32r, tag="wo", name="wo_sb")
            for l in range(L):
                for w_sb, w_d in ((wq_sb, wq_d), (wk_sb, wk_d), (wv_sb, wv_d)):
                    nc.sync.dma_start(
                        w_sb[:, l],
                        w_d[l].rearrange("(kc p) m -> p kc m", p=128),
                    )
                nc.sync.dma_start(wo_sb[:, l], wo_d[l])
            bq_sb = [
                wp.tile([1, 128], f32r, name=f"bqs{l}", tag=f"bq{l}")
                for l in range(L)
            ]
            for l in range(L):
                nc.sync.dma_start(bq_sb[l][:], bq_d[l : l + 1, :])
            ones_row = wp.tile([1, 512], f32r, tag="ones_row", name="ones_row")
            nc.sync.dma_start(ones_row[:], onesr_d[:])

            # broadcast rows (beta, gamma, B) to [128, 3L, D]
            rows_row = wp.tile([1, 3 * L, D], f32, tag="rows_row", name="rows_row")
            nc.sync.dma_start(
                rows_row[:], rows_d[:].rearrange("(o r) d -> o r d", o=1)
            )
            rows_bc = wp.tile([128, 3 * L, D], f32, tag="rows_bc", name="rows_bc")
            for r in range(3 * L):
                nc.gpsimd.partition_broadcast(rows_bc[:, r], rows_row[:, r])

            # --- state ---
            h_sb = st.tile([128, NT, D], f32, tag="h", name="h_sb")
            nc.sync.dma_start(h_sb[:], x_d[:].rearrange("(c p) d -> p c d", p=128))
            hT_sb = st.tile([128, NDT, S], f32r, tag="hT", name="hT_sb")
            qT_sb = st.tile([128, S], f32r, tag="qT", name="qT_sb")
            kT_sb = st.tile([128, S], f32r, tag="kT", name="kT_sb")
            vT_sb = st.tile([128, S], f16, tag="vT", name="vT_sb")
            v_sb = st.tile([128, NT, 128], f16, tag="v", name="v_sb")

            for l in range(L):
                # ---- transpose h -> hT (f32 transpose, copy casts to f32r) ----
                for dt in range(NDT):
                    for half in range(2):
                        tp = ps_w.tile([128, 512], f32, tag="work",
                                       name=f"tp{l}_{dt}_{half}")
                        for c in range(4):
                            ct = half * 4 + c
                            nc.tensor.transpose(
                                tp[:, c * 128 : (c + 1) * 128],
                                h_sb[:, ct, dt * 128 : (dt + 1) * 128],
                                ident[:],
                            )
                        nc.scalar.copy(
                            hT_sb[:, dt, half * 512 : (half + 1) * 512], tp[:]
                        )

                # ---- qT, kT, vT (f32r) ----
                for nh in range(2):
                    cs = slice(nh * 512, (nh + 1) * 512)
                    qp = ps_w.tile([128, 512], f32, tag="work", name=f"qp{l}_{nh}")
                    for dt in range(NDT):
                        nc.tensor.matmul(
                            qp[:], wq_sb[:, l, dt], hT_sb[:, dt, cs],
                            start=(dt == 0), stop=False,
                        )
                    nc.tensor.matmul(
                        qp[:], bq_sb[l][:], ones_row[:],
                        start=False, stop=True,
                    )
                    nc.scalar.copy(qT_sb[:, cs], qp[:])
                    kp = ps_w.tile([128, 512], f32, tag="work", name=f"kp{l}_{nh}")
                    for dt in range(NDT):
                        nc.tensor.matmul(
                            kp[:], wk_sb[:, l, dt], hT_sb[:, dt, cs],
                            start=(dt == 0), stop=(dt == NDT - 1),
                        )
                    nc.scalar.copy(kT_sb[:, cs], kp[:])
                    vp = ps_w.tile([128, 512], f32, tag="work", name=f"vp{l}_{nh}")
                    for dt in range(NDT):
                        nc.tensor.matmul(
                            vp[:], wv_sb[:, l, dt], hT_sb[:, dt, cs],
                            start=(dt == 0), stop=(dt == NDT - 1),
                        )
                    nc.scalar.copy(vT_sb[:, cs], vp[:])  # cast fp16
                # v = vT.T per token tile (fp16 transposes)
                vtp = ps_p.tile([128, NT, 128], f16, tag="pt", name=f"vtp{l}")
                for c in range(NT):
                    nc.tensor.transpose(
                        vtp[:, c], vT_sb[:, c * 128 : (c + 1) * 128], ident16[:]
                    )
                nc.vector.tensor_copy(v_sb[:], vtp[:])

                # ---- attention (qt-outer for split AllReduce) ----
                for qt in range(NT):
                    hf, r4 = divmod(qt, 4)
                    oT_ps = ps_o.tile([128, 128], f32, tag="oT",
                                      name=f"oT{l}_{qt}")
                    iz2 = sm.tile([128, 2], f32, tag="iz2", name=f"iz2{l}_{qt}")
                    for h in range(2):
                        hs = slice(h * 64, (h + 1) * 64)
                        e = sb.tile([128, S], f16, tag="e", name=f"e{l}_{qt}_{h}")
                        for nh in range(2):
                            cs = slice(nh * 512, (nh + 1) * 512)
                            s_ps = ps_s.tile(
                                [128, 512], f32, tag="s",
                                name=f"sps{l}_{qt}_{h}_{nh}",
                            )
                            nc.tensor.matmul(
                                s_ps[:],
                                qT_sb[hs, qt * 128 : (qt + 1) * 128],
                                kT_sb[hs, cs],
                                start=True, stop=True,
                            )
                            nc.scalar.activation(
                                e[:, cs], s_ps[:], AF.Exp, scale=float(SCALE)
                            )
                        m8a = sm.tile([128, 8], f32, tag="m8a",
                                      name=f"m8a{l}_{qt}_{h}")
                        e2 = sb.tile([128, S], f16, tag="e2",
                                     name=f"e2{l}_{qt}_{h}")
                        m8b = sm.tile([128, 8], f32, tag="m8b",
                                      name=f"m8b{l}_{qt}_{h}")
                        nc.vector.max(m8a[:], e[:])
                        nc.vector._custom_dve(
                            SELZ, out=e2[:], in0=e[:], s0=m8a[:, 7:8]
                        )
                        nc.vector.max(m8b[:], e2[:])
                        p = sb.tile([128, S], f16, tag="p", name=f"p{l}_{qt}_{h}")
                        nc.vector._custom_dve(
                            SELGE, out=p[:], in0=e[:], s0=m8b[:, 7:8]
                        )
                        pT_ps = ps_p.tile([128, NT, 128], f16, tag="pt",
                                          name=f"pT{l}_{qt}_{h}")
                        for kc in range(NT):
                            nc.tensor.transpose(
                                pT_ps[:, kc], p[:, kc * 128 : (kc + 1) * 128],
                                ident16[:],
                            )
                        pT = sb.tile([128, NT, 128], f16, tag="pT",
                                     name=f"pTs{l}_{qt}_{h}")
                        if (qt + h) % 2 == 0:
                            nc.vector.tensor_copy(pT[:], pT_ps[:])
                        else:
                            nc.scalar.copy(pT[:], pT_ps[:])
                        for kc in range(NT):
                            nc.tensor.matmul(
                                oT_ps[hs, :],
                                v_sb[:, kc, hs],
                                pT[:, kc],
                                start=(kc == 0), stop=(kc == NT - 1),
                            )
                        # Z = sum of the 16 kept exps (off the critical path;
                        # normalization is applied at the y projection)
                        dm = sm.tile([128, 8], f32, tag="dm",
                                     name=f"dm{l}_{qt}_{h}")
                        za = sm.tile([128, 1], f32, tag="za",
                                     name=f"za{l}_{qt}_{h}")
                        nc.scalar.activation(dm[:], m8a[:], AF.Copy,
                                             accum_out=za[:])
                        dmb = sm.tile([128, 8], f32, tag="dmb",
                                      name=f"dmb{l}_{qt}_{h}")
                        zb = sm.tile([128, 1], f32, tag="zb",
                                     name=f"zb{l}_{qt}_{h}")
                        nc.scalar.activation(dmb[:], m8b[:], AF.Copy,
                                             accum_out=zb[:])
                        zs = sm.tile([128, 1], f32, tag="zs",
                                     name=f"zs{l}_{qt}_{h}")
                        nc.vector.tensor_add(zs[:], za[:], zb[:])
                        nc.vector.reciprocal(iz2[:, h : h + 1], zs[:])
                    # y[qt] per head, scaled by 1/Z at the PSUM copy
                    oT_sb = sb.tile([128, 128], f32r, tag="oTsb",
                                    name=f"oTsb{l}_{qt}")
                    nc.scalar.copy(oT_sb[:], oT_ps[:])
                    ys = []
                    for h in range(2):
                        hs = slice(h * 64, (h + 1) * 64)
                        y_ps = ps_w.tile([128, 512], f32, tag="work",
                                         name=f"yps{l}_{qt}_{h}")
                        nc.tensor.matmul(
                            y_ps[:], oT_sb[hs, :], wo_sb[hs, l],
                            start=True, stop=True,
                        )
                        yh = sb.tile([128, 512], f32, tag=f"ysc{h}",
                                     name=f"ysc{l}_{qt}_{h}")
                        nc.scalar.activation(
                            yh[:], y_ps[:], AF.Copy, scale=iz2[:, h : h + 1],
                        )
                        ys.append(yh)
                    y_sb = sb.tile([128, D], bf16, tag="y_sb",
                                   name=f"ysb{l}_{qt}")
                    nc.vector.tensor_add(y_sb[:], ys[0][:], ys[1][:])
                    nc.sync.dma_start(
                        cc_in[l][hf][r4 * 128 : (r4 + 1) * 128, :], y_sb[:]
                    )
                    if qt == 3 or qt == NT - 1:
                        hf = 0 if qt == 3 else 1
                        if sim:
                            for r in range(4):
                                cpt = sb.tile([128, D], bf16, tag="cp",
                                              name=f"cp{l}_{hf}_{r}")
                                nc.sync.dma_start(
                                    cpt[:], cc_in[l][hf][r * 128 : (r + 1) * 128, :]
                                )
                                nc.sync.dma_start(
                                    cc_out[l][hf][r * 128 : (r + 1) * 128, :], cpt[:]
                                )
                        else:
                            nc.gpsimd.collective_compute(
                                "AllReduce",
                                mybir.AluOpType.add,
                                replica_groups=GROUPS,
                                ins=[cc_in[l][hf][:]],
                                outs=[cc_out[l][hf][:]],
                            )

                # ---- residual + bias + LN (redundant on every core) ----
                for t in range(NT):
                    hf, r = divmod(t, 4)
                    yt = sb.tile([128, D], bf16, tag="yt", name=f"yt{l}_{t}")
                    nc.sync.dma_start(
                        yt[:], cc_out[l][hf][r * 128 : (r + 1) * 128, :]
                    )
                    y1 = sb.tile([128, D], f32, tag="y1", name=f"y1{l}_{t}")
                    nc.vector.tensor_add(y1[:], yt[:], h_sb[:, t, :])
                    nc.gpsimd.tensor_add(y1[:], y1[:], rows_bc[:, 2 * L + l])
                    stats = sm.tile([128, 6], f32, tag="stats", name=f"st{l}_{t}")
                    nc.vector.bn_stats(stats[:], y1[:])
                    mv = sm.tile([128, 2], f32, tag="mv", name=f"mv{l}_{t}")
                    nc.vector.bn_aggr(mv[:], stats[:])
                    std = sm.tile([128, 1], f32, tag="std", name=f"sd{l}_{t}")
                    nc.scalar.activation(
                        std[:], mv[:, 1:2], AF.Sqrt, scale=float(D / (D - 1))
                    )
                    nc.vector.tensor_scalar_add(std[:], std[:], float(EPS))
                    rstd = sm.tile([128, 1], f32, tag="rstd", name=f"rs{l}_{t}")
                    nc.vector.reciprocal(rstd[:], std[:])
                    hh = sb.tile([128, D], f32, tag="hh", name=f"hh{l}_{t}")
                    nc.vector.tensor_scalar(
                        hh[:], y1[:], mv[:, 0:1], rstd[:],
                        op0=AL.subtract, op1=AL.mult,
                    )
                    hn = sb.tile([128, D], f32, tag="hn", name=f"hn{l}_{t}")
                    nc.gpsimd.tensor_mul(hn[:], hh[:], rows_bc[:, l])
                    nc.gpsimd.tensor_add(h_sb[:, t, :], hn[:], rows_bc[:, L + l])
                    if l == L - 1:
                        nc.sync.dma_start(
                            out_d[t * 128 : (t + 1) * 128, :], h_sb[:, t, :]
                        )

    nc.compile()
    return nc


def _get_compiled():
    global _COMPILED
    if _COMPILED is None:
        _COMPILED = _build()
    return _COMPILED


def _host_prep(x, Wq, Wk, Wv, Wo, bq, bk, bv, bo, gamma, beta):
    """Build the 8 per-core input maps."""
    Bv_Wo = np.stack([bv[l] @ Wo[l] + bo[l] for l in range(L)])  # [L, D]
    rows = np.concatenate([beta, gamma, Bv_Wo], axis=0).astype(np.float32)
    in_maps = []
    for c in range(8):
        b, r = divmod(c, 4)
        cols = slice(128 * r, 128 * (r + 1))
        in_maps.append(
            {
                "x": np.ascontiguousarray(x[b]).astype(np.float32),
                "wq": np.ascontiguousarray(Wq[:, :, cols]).astype(np.float32),
                "wk": np.ascontiguousarray(Wk[:, :, cols]).astype(np.float32),
                "wv": np.ascontiguousarray(Wv[:, :, cols]).astype(np.float32),
                "wo": np.ascontiguousarray(Wo[:, cols, :]).astype(np.float32),
                "bq": np.ascontiguousarray(bq[:, cols]).astype(np.float32),
                "onesr": np.ones((1, 512), np.float32),
                "rows": rows,
            }
        )
    return in_maps


def _numpy_fallback(x, mask, Wq, Wk, Wv, Wo, bq, bk, bv, bo, gamma, beta):
    m = np.asarray(mask)[:, None, :, :]
    h = np.asarray(x, dtype=np.float64)
    for l in range(L):
        q = (h @ Wq[l] + bq[l]).reshape(B, S, H, DK).transpose(0, 2, 1, 3)
        k = (h @ Wk[l] + bk[l]).reshape(B, S, H, DK).transpose(0, 2, 1, 3)
        v = (h @ Wv[l] + bv[l]).reshape(B, S, H, DK).transpose(0, 2, 1, 3)
        s = np.einsum("bhqd,bhkd->bhqk", q, k) * SCALE
        kth = np.sort(s, axis=-1)[..., -TOPK][..., None]
        keep = (s >= kth) & m
        sm = np.where(keep, s, -1e9)
        sm = sm - sm.max(-1, keepdims=True)
        p = np.exp(sm)
        p /= p.sum(-1, keepdims=True)
        o = np.einsum("bhqk,bhkd->bhqd", p, v)
        o = o.transpose(0, 2, 1, 3).reshape(B, S, D) @ Wo[l] + bo[l]
        y = h + o
        mean = y.mean(-1, keepdims=True)
        std = y.std(-1, ddof=1, keepdims=True)
        h = beta[l] * (y - mean) / (std + EPS) + gamma[l]
    return h.astype(np.float32)


def kernel(x, mask, Wq, Wk, Wv, Wo, bq, bk, bv, bo, gamma, beta):
    x = np.asarray(x, dtype=np.float32)
    mask_np = np.asarray(mask)
    args = [
        np.asarray(a, dtype=np.float32)
        for a in (Wq, Wk, Wv, Wo, bq, bk, bv, bo, gamma, beta)
    ]
    if not mask_np.all():
        return _numpy_fallback(x, mask_np, *args)

    from concourse import bass_utils

    nc = _get_compiled()
    in_maps = _host_prep(x, *args)
    res = bass_utils.run_bass_kernel_spmd(nc, in_maps, core_ids=list(range(8)))
    out = np.stack([res.results[0]["out"], res.results[4]["out"]])
    return out.astype(np.float32)



# revision 5
# speedup vs baseline: 1.1802x; 1.1802x over previous
"""Trainium2 Bass kernel v3 for nn_Encoder_17918603559377 (4-layer sparse-attention
encoder, top-16 per row, B=2 S=1024 D=512 H=8).

Sharding: 8 cores; core c handles batch c//4, heads {2r, 2r+1} where r = c%4.
Per layer the partial y = o @ Wo are AllReduce-summed (bf16) in three token
segments (tiles 0-3, 4-5, 6-7) so each AR overlaps trailing attention compute
or next-layer prep.

Key structure vs v2:
- LayerNorm affine (beta mult / gamma add) is folded into the next layer's
  weights host-side: the kernel keeps the *normalized* state n and a residual
  carrier r = beta*n + C (C = gamma + B_next), computed on GpSimd off the
  critical path. Next-layer transposes/QKV consume n directly.
- Per-layer epilogue is software-pipelined: LN tiles 0-3 -> transposes half 0
  -> QKV nh=0 run while the tail ARs are still in flight; LN tiles 4-7 ->
  transposes half 1 -> QKV nh=1 follow.
- Top-16 selection: max8 + scalar_tensor_tensor selects ((e<t)*e, (e>=t)*e)
  with the softmax denominator Z taken from the select's accum_out.
- Weights/x are host-packed partition-major so every input DMA is contiguous.
- A dummy warmup AllReduce absorbs the ~40us first-collective cost.
"""

import sys

sys.path.insert(0, "/opt/trn_rl_repo")

import numpy as np

L, B, S, D, H, DK = 4, 2, 1024, 512, 8, 64
TOPK = 16
EPS = 1e-6
SCALE = 1.0 / np.sqrt(DK)
NT = S // 128  # token tiles
NDT = D // 128  # d-dim tiles

# AllReduce token segments: tiles [0,4), [4,6), [6,8)
SEGS = [(0, 4), (4, 6), (6, 8)]

_COMPILED = None


def _build(sim=False):
    import concourse.bacc as bacc
    import concourse.mybir as mybir
    import concourse.tile as tile
    from concourse import masks

    f32 = mybir.dt.float32
    f32r = mybir.dt.float32r
    f16 = mybir.dt.float16
    bf16 = mybir.dt.bfloat16
    AL = mybir.AluOpType
    AF = mybir.ActivationFunctionType

    nc = bacc.Bacc(
        "TRN2", target_bir_lowering=False, debug=False,
        num_devices=(1 if sim else 8),
    )

    # host-packed partition-major inputs (all contiguous DMAs)
    x_d = nc.dram_tensor("x", (128, NT * D), f32, kind="ExternalInput")
    xr_d = nc.dram_tensor("xr", (128, NT * D), f32, kind="ExternalInput")
    wq_d = nc.dram_tensor("wq", (128, L * NDT * 128), f32r, kind="ExternalInput")
    wk_d = nc.dram_tensor("wk", (128, L * NDT * 128), f32r, kind="ExternalInput")
    wv_d = nc.dram_tensor("wv", (128, L * NDT * 128), f32r, kind="ExternalInput")
    wo_d = nc.dram_tensor("wo", (128, L * D), f32r, kind="ExternalInput")
    bq_d = nc.dram_tensor("bq", (L, 128), f32r, kind="ExternalInput")
    onesr_d = nc.dram_tensor("onesr", (1, 512), f32r, kind="ExternalInput")
    # rows: [0:L] beta, [L:2L] C[l] (= gamma[l] + B[l+1]; C[L-1] = gamma[L-1])
    rows_d = nc.dram_tensor("rows", (2 * L, D), f32, kind="ExternalInput")
    out_d = nc.dram_tensor("out", (S, D), f32, kind="ExternalOutput")

    cc_in = [
        [nc.dram_tensor(f"cc_in{l}_{s}", ((b - a) * 128, D), bf16, kind="Internal")
         for s, (a, b) in enumerate(SEGS)] for l in range(L)
    ]
    cc_out = [
        [nc.dram_tensor(f"cc_out{l}_{s}", ((b - a) * 128, D), bf16, kind="Internal")
         for s, (a, b) in enumerate(SEGS)] for l in range(L)
    ]
    ccw_in = nc.dram_tensor("ccw_in", (128, 64), bf16, kind="Internal")
    ccw_out = nc.dram_tensor("ccw_out", (128, 64), bf16, kind="Internal")
    GROUPS = [[0, 1, 2, 3], [4, 5, 6, 7]]

    with tile.TileContext(nc) as tc:
        with (
            tc.tile_pool(name="w", bufs=1) as wp,
            tc.tile_pool(name="state", bufs=1) as st,
            tc.tile_pool(name="sb", bufs=3) as sb,
            tc.tile_pool(name="sm", bufs=3) as sm,
            tc.tile_pool(name="ps_w", bufs=2, space="PSUM") as ps_w,
            tc.tile_pool(name="ps_s", bufs=2, space="PSUM") as ps_s,
            tc.tile_pool(name="ps_p", bufs=2, space="PSUM") as ps_p,
            tc.tile_pool(name="ps_o", bufs=2, space="PSUM") as ps_o,
        ):
            # ---- warmup collective (absorbs first-CC latency) ----
            if not sim:
                nc.gpsimd.collective_compute(
                    "AllReduce", mybir.AluOpType.add,
                    replica_groups=GROUPS,
                    ins=[ccw_in[:]], outs=[ccw_out[:]],
                )

            ident = wp.tile([128, 128], f32, tag="ident", name="ident")
            masks.make_identity(nc, ident[:])
            ident16 = wp.tile([128, 128], f16, tag="ident16", name="ident16")
            nc.scalar.copy(ident16[:], ident[:])

            # ---- state ----
            x_sb = st.tile([128, NT, D], f32, tag="x", name="x_sb")
            r_sb = st.tile([128, NT, D], f32, tag="r", name="r_sb")
            n_sb = st.tile([128, NT, D], f32, tag="n", name="n_sb")
            hT_sb = st.tile([128, NDT, S], f32r, tag="hT", name="hT_sb")
            qT_sb = st.tile([128, S], f32r, tag="qT", name="qT_sb")
            kT_sb = st.tile([128, S], f32r, tag="kT", name="kT_sb")
            vT_sb = st.tile([128, S], f16, tag="vT", name="vT_sb")
            v_sb = st.tile([128, NT, 128], f16, tag="v", name="v_sb")

            # ---- weight tiles ----
            wq_sb = wp.tile([128, L, NDT, 128], f32r, tag="wq", name="wq_sb")
            wk_sb = wp.tile([128, L, NDT, 128], f32r, tag="wk", name="wk_sb")
            wv_sb = wp.tile([128, L, NDT, 128], f32r, tag="wv", name="wv_sb")
            wo_sb = wp.tile([128, L, D], f32r, tag="wo", name="wo_sb")
            bq_sb = [
                wp.tile([1, 128], f32r, name=f"bqs{l}", tag=f"bq{l}")
                for l in range(L)
            ]
            ones_row = wp.tile([1, 512], f32r, tag="ones_row", name="ones_row")
            rows_row = wp.tile([1, 2 * L, D], f32, tag="rows_row", name="rows_row")
            rows_bc = wp.tile([128, 2 * L, D], f32, tag="rows_bc", name="rows_bc")

            # ---- DMA: first-needed first; spread across engine queues ----
            nc.sync.dma_start(x_sb[:], x_d[:].rearrange("p (c d) -> p c d", c=NT))
            LW = NDT * 128  # per-layer columns in packed w tensors
            for w_sb, w_d in ((wq_sb, wq_d), (wk_sb, wk_d), (wv_sb, wv_d)):
                nc.sync.dma_start(
                    w_sb[:, 0], w_d[:, 0:LW].rearrange("p (kc m) -> p kc m", kc=NDT)
                )
            nc.sync.dma_start(wo_sb[:, 0], wo_d[:, 0:D])
            nc.scalar.dma_start(
                r_sb[:], xr_d[:].rearrange("p (c d) -> p c d", c=NT)
            )
            nc.scalar.dma_start(ones_row[:], onesr_d[:])
            for l in range(L):
                nc.scalar.dma_start(bq_sb[l][:], bq_d[l : l + 1, :])
            nc.gpsimd.dma_start(
                rows_row[:], rows_d[:].rearrange("(o r) d -> o r d", o=1)
            )
            for l in range(1, L):
                for eng, (w_sb, w_d) in zip(
                    (nc.gpsimd, nc.gpsimd, nc.scalar),
                    ((wq_sb, wq_d), (wk_sb, wk_d), (wv_sb, wv_d)),
                ):
                    eng.dma_start(
                        w_sb[:, l],
                        w_d[:, l * LW : (l + 1) * LW].rearrange(
                            "p (kc m) -> p kc m", kc=NDT
                        ),
                    )
                nc.gpsimd.dma_start(wo_sb[:, l], wo_d[:, l * D : (l + 1) * D])
            for r in range(2 * L):
                nc.gpsimd.partition_broadcast(rows_bc[:, r], rows_row[:, r])

            # ---------------- helpers ----------------
            def prep_half(l, src_sb, half):
                """Transposes of token tiles [half*4, half*4+4) + QKV nh=half
                (+ v transposes of this half's key tiles)."""
                cs = slice(half * 512, (half + 1) * 512)
                for dt in range(NDT):
                    tp = ps_w.tile([128, 512], f32, tag="work",
                                   name=f"tp{l}_{dt}_{half}")
                    for c in range(4):
                        ct = half * 4 + c
                        nc.tensor.transpose(
                            tp[:, c * 128 : (c + 1) * 128],
                            src_sb[:, ct, dt * 128 : (dt + 1) * 128],
                            ident[:],
                        )
                    nc.scalar.copy(hT_sb[:, dt, cs], tp[:])
                qp = ps_w.tile([128, 512], f32, tag="work", name=f"qp{l}_{half}")
                for dt in range(NDT):
                    nc.tensor.matmul(
                        qp[:], wq_sb[:, l, dt], hT_sb[:, dt, cs],
                        start=(dt == 0), stop=False,
                    )
                nc.tensor.matmul(
                    qp[:], bq_sb[l][:], ones_row[:], start=False, stop=True
                )
                nc.scalar.copy(qT_sb[:, cs], qp[:])
                kp = ps_w.tile([128, 512], f32, tag="work", name=f"kp{l}_{half}")
                for dt in range(NDT):
                    nc.tensor.matmul(
                        kp[:], wk_sb[:, l, dt], hT_sb[:, dt, cs],
                        start=(dt == 0), stop=(dt == NDT - 1),
                    )
                nc.scalar.copy(kT_sb[:, cs], kp[:])
                vp = ps_w.tile([128, 512], f32, tag="work", name=f"vp{l}_{half}")
                for dt in range(NDT):
                    nc.tensor.matmul(
                        vp[:], wv_sb[:, l, dt], hT_sb[:, dt, cs],
                        start=(dt == 0), stop=(dt == NDT - 1),
                    )
                nc.scalar.copy(vT_sb[:, cs], vp[:])  # cast fp16
                vtp = ps_p.tile([128, 4, 128], f16, tag="pt",
                                name=f"vtp{l}_{half}")
                for c in range(4):
                    kc = half * 4 + c
                    nc.tensor.transpose(
                        vtp[:, c], vT_sb[:, kc * 128 : (kc + 1) * 128], ident16[:]
                    )
                nc.vector.tensor_copy(
                    v_sb[:, half * 4 : half * 4 + 4], vtp[:]
                )

            def attention(l):
                for qt in range(NT):
                    oT_ps = ps_o.tile([128, 128], f32, tag="oT",
                                      name=f"oT{l}_{qt}")
                    z2 = sm.tile([128, 2], f32, tag="z2", name=f"z2{l}_{qt}")
                    iz2 = sm.tile([128, 2], f32, tag="iz2", name=f"iz2{l}_{qt}")
                    for h in range(2):
                        hs = slice(h * 64, (h + 1) * 64)
                        e = sb.tile([128, S], f16, tag="e", name=f"e{l}_{qt}_{h}")
                        for nh in range(2):
                            cs = slice(nh * 512, (nh + 1) * 512)
                            s_ps = ps_s.tile(
                                [128, 512], f32, tag="s",
                                name=f"sps{l}_{qt}_{h}_{nh}",
                            )
                            nc.tensor.matmul(
                                s_ps[:],
                                qT_sb[hs, qt * 128 : (qt + 1) * 128],
                                kT_sb[hs, cs],
                                start=True, stop=True,
                            )
                            nc.scalar.activation(
                                e[:, cs], s_ps[:], AF.Exp, scale=float(SCALE)
                            )
                        m8a = sm.tile([128, 8], f32, tag="m8a",
                                      name=f"m8a{l}_{qt}_{h}")
                        nc.vector.max(m8a[:], e[:])
                        e2 = sb.tile([128, S], f16, tag="e2",
                                     name=f"e2{l}_{qt}_{h}")
                        nc.vector.scalar_tensor_tensor(
                            e2[:], e[:], m8a[:, 7:8], e[:],
                            op0=AL.is_lt, op1=AL.mult,
                        )
                        m8b = sm.tile([128, 8], f32, tag="m8b",
                                      name=f"m8b{l}_{qt}_{h}")
                        nc.vector.max(m8b[:], e2[:])
                        p = sb.tile([128, S], f16, tag="p", name=f"p{l}_{qt}_{h}")
                        nc.vector.scalar_tensor_tensor(
                            p[:], e[:], m8b[:, 7:8], e[:],
                            op0=AL.is_ge, op1=AL.mult,
                            accum_out=z2[:, h : h + 1],
                        )
                        pT_ps = ps_p.tile([128, NT, 128], f16, tag="pt",
                                          name=f"pT{l}_{qt}_{h}")
                        for kc in range(NT):
                            nc.tensor.transpose(
                                pT_ps[:, kc], p[:, kc * 128 : (kc + 1) * 128],
                                ident16[:],
                            )
                        pT = sb.tile([128, NT, 128], f16, tag="pT",
                                     name=f"pTs{l}_{qt}_{h}")
                        if (qt + h) % 2 == 0:
                            nc.vector.tensor_copy(pT[:], pT_ps[:])
                        else:
                            nc.scalar.copy(pT[:], pT_ps[:])
                        for kc in range(NT):
                            nc.tensor.matmul(
                                oT_ps[hs, :],
                                v_sb[:, kc, hs],
                                pT[:, kc],
                                start=(kc == 0), stop=(kc == NT - 1),
                            )
                    nc.vector.reciprocal(iz2[:], z2[:])
                    oT_sb = sb.tile([128, 128], f32r, tag="oTsb",
                                    name=f"oTsb{l}_{qt}")
                    nc.scalar.copy(oT_sb[:], oT_ps[:])
                    ys = []
                    for h in range(2):
                        hs = slice(h * 64, (h + 1) * 64)
                        y_ps = ps_w.tile([128, 512], f32, tag="work",
                                         name=f"yps{l}_{qt}_{h}")
                        nc.tensor.matmul(
                            y_ps[:], oT_sb[hs, :], wo_sb[hs, l],
                            start=True, stop=True,
                        )
                        yh = sb.tile([128, 512], f32, tag=f"ysc{h}",
                                     name=f"ysc{l}_{qt}_{h}")
                        nc.scalar.activation(
                            yh[:], y_ps[:], AF.Copy, scale=iz2[:, h : h + 1],
                        )
                        ys.append(yh)
                    y_sb = sb.tile([128, D], bf16, tag="y_sb",
                                   name=f"ysb{l}_{qt}")
                    nc.vector.tensor_add(y_sb[:], ys[0][:], ys[1][:])
                    seg = 0 if qt < 4 else (1 if qt < 6 else 2)
                    row0 = (qt - SEGS[seg][0]) * 128
                    nc.sync.dma_start(
                        cc_in[l][seg][row0 : row0 + 128, :], y_sb[:]
                    )
                    if qt in (3, 5, 7):
                        seg = {3: 0, 5: 1, 7: 2}[qt]
                        if sim:
                            nt_seg = SEGS[seg][1] - SEGS[seg][0]
                            for rr in range(nt_seg):
                                cpt = sb.tile([128, D], bf16, tag="cp",
                                              name=f"cp{l}_{seg}_{rr}")
                                nc.sync.dma_start(
                                    cpt[:],
                                    cc_in[l][seg][rr * 128 : (rr + 1) * 128, :],
                                )
                                nc.sync.dma_start(
                                    cc_out[l][seg][rr * 128 : (rr + 1) * 128, :],
                                    cpt[:],
                                )
                        else:
                            nc.gpsimd.collective_compute(
                                "AllReduce",
                                mybir.AluOpType.add,
                                replica_groups=GROUPS,
                                ins=[cc_in[l][seg][:]],
                                outs=[cc_out[l][seg][:]],
                            )

            def ln_tile(l, t):
                """Residual + LayerNorm -> n_sb[t] (normalized, no affine)."""
                seg = 0 if t < 4 else (1 if t < 6 else 2)
                row0 = (t - SEGS[seg][0]) * 128
                yt = sb.tile([128, D], bf16, tag="yt", name=f"yt{l}_{t}")
                nc.sync.dma_start(yt[:], cc_out[l][seg][row0 : row0 + 128, :])
                y1 = sb.tile([128, D], f32, tag="y1", name=f"y1{l}_{t}")
                nc.vector.tensor_add(y1[:], yt[:], r_sb[:, t, :])
                stats = sm.tile([128, 6], f32, tag="stats", name=f"st{l}_{t}")
                nc.vector.bn_stats(stats[:], y1[:])
                mv = sm.tile([128, 2], f32, tag="mv", name=f"mv{l}_{t}")
                nc.vector.bn_aggr(mv[:], stats[:])
                # rstd ~ 1/(std+eps) ~ 1/sqrt(var * D/(D-1))  (eps=1e-6 negligible)
                std = sm.tile([128, 1], f32, tag="std", name=f"sd{l}_{t}")
                nc.scalar.activation(
                    std[:], mv[:, 1:2], AF.Sqrt, scale=float(D / (D - 1))
                )
                rstd = sm.tile([128, 1], f32, tag="rstd", name=f"rs{l}_{t}")
                nc.vector.reciprocal(rstd[:], std[:])
                nc.vector.tensor_scalar(
                    n_sb[:, t, :], y1[:], mv[:, 0:1], rstd[:],
                    op0=AL.subtract, op1=AL.mult,
                )

            def r_tile(l, t):
                """r = beta_l * n + C_l (GpSimd, off critical path).
                For the last layer r IS the output."""
                nc.gpsimd.tensor_mul(r_sb[:, t, :], n_sb[:, t, :], rows_bc[:, l])
                nc.gpsimd.tensor_add(r_sb[:, t, :], r_sb[:, t, :],
                                     rows_bc[:, L + l])
                if l == L - 1:
                    nc.sync.dma_start(
                        out_d[t * 128 : (t + 1) * 128, :], r_sb[:, t, :]
                    )

            # ---------------- main schedule ----------------
            prep_half(0, x_sb, 0)
            prep_half(0, x_sb, 1)
            for l in range(L):
                attention(l)
                for half in range(2):
                    for t in range(half * 4, half * 4 + 4):
                        ln_tile(l, t)
                    if l < L - 1:
                        prep_half(l + 1, n_sb, half)
                for t in range(NT):
                    r_tile(l, t)

    nc.compile()
    return nc


def _get_compiled():
    global _COMPILED
    if _COMPILED is None:
        _COMPILED = _build()
    return _COMPILED


def _host_prep(x, Wq, Wk, Wv, Wo, bq, bk, bv, bo, gamma, beta):
    """Build the 8 per-core input maps (all arrays partition-major packed)."""
    f64 = np.float64
    Wq, Wk, Wv, Wo = [a.astype(f64) for a in (Wq, Wk, Wv, Wo)]
    bq, bk, bv, bo, gamma, beta = [
        a.astype(f64) for a in (bq, bk, bv, bo, gamma, beta)
    ]
    # fold previous-layer LN affine into weights:
    # l=0 consumes raw x; l>=1 consumes n^{l-1}:
    #   h = beta_{l-1} * n + gamma_{l-1}
    #   h @ W = n @ (diag(beta)W) + gamma @ W
    Wq_e = np.stack([Wq[l] if l == 0 else beta[l - 1][:, None] * Wq[l]
                     for l in range(L)])
    Wk_e = np.stack([Wk[l] if l == 0 else beta[l - 1][:, None] * Wk[l]
                     for l in range(L)])
    Wv_e = np.stack([Wv[l] if l == 0 else beta[l - 1][:, None] * Wv[l]
                     for l in range(L)])
    bq_e = np.stack([bq[l] if l == 0 else bq[l] + gamma[l - 1] @ Wq[l]
                     for l in range(L)])
    # bk dropped: adds a per-query-row constant to scores (softmax/topk
    # invariant). bv folded into the bias row B below.
    bv_e = np.stack([bv[l] if l == 0 else bv[l] + gamma[l - 1] @ Wv[l]
                     for l in range(L)])
    Bl = np.stack([bv_e[l] @ Wo[l] + bo[l] for l in range(L)])  # [L, D]
    # residual carrier constants: y_{l+1} = o@Wo + B[l+1] + beta_l*n + gamma_l
    C = np.stack([gamma[l] + (Bl[l + 1] if l + 1 < L else 0.0)
                  for l in range(L)])
    rows = np.concatenate([beta, C], axis=0).astype(np.float32)  # [2L, D]

    in_maps = []
    for c in range(8):
        b, r = divmod(c, 4)
        cols = slice(128 * r, 128 * (r + 1))
        # partition-major packs
        xb = x[b].astype(f64)  # [S, D]
        x_pm = np.ascontiguousarray(
            xb.reshape(NT, 128, D).transpose(1, 0, 2).reshape(128, NT * D)
        ).astype(np.float32)
        xr_pm = np.ascontiguousarray(
            (xb + Bl[0]).reshape(NT, 128, D).transpose(1, 0, 2).reshape(
                128, NT * D)
        ).astype(np.float32)

        def pack_w(W_e):
            # [L, D, 128cols] -> [128p, L*NDT*128m]; p = contract row % 128
            wl = W_e[:, :, cols]  # [L, D, 128]
            return np.ascontiguousarray(
                wl.reshape(L, NDT, 128, 128).transpose(2, 0, 1, 3).reshape(
                    128, L * NDT * 128)
            ).astype(np.float32)

        wo_pm = np.ascontiguousarray(
            Wo[:, cols, :].transpose(1, 0, 2).reshape(128, L * D)
        ).astype(np.float32)
        in_maps.append(
            {
                "x": x_pm,
                "xr": xr_pm,
                "wq": pack_w(Wq_e),
                "wk": pack_w(Wk_e),
                "wv": pack_w(Wv_e),
                "wo": wo_pm,
                "bq": np.ascontiguousarray(bq_e[:, cols]).astype(np.float32),
                "onesr": np.ones((1, 512), np.float32),
                "rows": rows,
            }
        )
    return in_maps


def _numpy_fallback(x, mask, Wq, Wk, Wv, Wo, bq, bk, bv, bo, gamma, beta):
    m = np.asarray(mask)[:, None, :, :]
    h = np.asarray(x, dtype=np.float64)
    for l in range(L):
        q = (h @ Wq[l] + bq[l]).reshape(B, S, H, DK).transpose(0, 2, 1, 3)
        k = (h @ Wk[l] + bk[l]).reshape(B, S, H, DK).transpose(0, 2, 1, 3)
        v = (h @ Wv[l] + bv[l]).reshape(B, S, H, DK).transpose(0, 2, 1, 3)
        s = np.einsum("bhqd,bhkd->bhqk", q, k) * SCALE
        kth = np.sort(s, axis=-1)[..., -TOPK][..., None]
        keep = (s >= kth) & m
        sm = np.where(keep, s, -1e9)
        sm = sm - sm.max(-1, keepdims=True)
        p = np.exp(sm)
        p /= p.sum(-1, keepdims=True)
        o = np.einsum("bhqk,bhkd->bhqd", p, v)
        o = o.transpose(0, 2, 1, 3).reshape(B, S, D) @ Wo[l] + bo[l]
        y = h + o
        mean = y.mean(-1, keepdims=True)
        std = y.std(-1, ddof=1, keepdims=True)
        h = beta[l] * (y - mean) / (std + EPS) + gamma[l]
    return h.astype(np.float32)


def kernel(x, mask, Wq, Wk, Wv, Wo, bq, bk, bv, bo, gamma, beta):
    x = np.asarray(x, dtype=np.float32)
    mask_np = np.asarray(mask)
    args = [
        np.asarray(a, dtype=np.float32)
        for a in (Wq, Wk, Wv, Wo, bq, bk, bv, bo, gamma, beta)
    ]
    if not mask_np.all():
        return _numpy_fallback(x, mask_np, *args)

    from concourse import bass_utils

    nc = _get_compiled()
    in_maps = _host_prep(x, *args)
    res = bass_utils.run_bass_kernel_spmd(nc, in_maps, core_ids=list(range(8)))
    out = np.stack([res.results[0]["out"], res.results[4]["out"]])
    return out.astype(np.float32)


# revision 28
# speedup vs baseline: 1.3496x; 1.1436x over previous
"""Trainium2 Bass kernel v3 for nn_Encoder_17918603559377 (4-layer sparse-attention
encoder, top-16 per row, B=2 S=1024 D=512 H=8).

Sharding: 8 cores; core c handles batch c//4, heads {2r, 2r+1} where r = c%4.
Per layer the partial y = o @ Wo are AllReduce-summed (bf16) in three token
segments (tiles 0-3, 4-5, 6-7) so each AR overlaps trailing attention compute
or next-layer prep.

Key structure vs v2:
- LayerNorm affine (beta mult / gamma add) is folded into the next layer's
  weights host-side: the kernel keeps the *normalized* state n and a residual
  carrier r = beta*n + C (C = gamma + B_next), computed on GpSimd off the
  critical path. Next-layer transposes/QKV consume n directly.
- Per-layer epilogue is software-pipelined: LN tiles 0-3 -> transposes half 0
  -> QKV nh=0 run while the tail ARs are still in flight; LN tiles 4-7 ->
  transposes half 1 -> QKV nh=1 follow.
- Top-16 selection: max8 + scalar_tensor_tensor selects ((e<t)*e, (e>=t)*e)
  with the softmax denominator Z taken from the select's accum_out.
- Weights/x are host-packed partition-major so every input DMA is contiguous.
- A dummy warmup AllReduce absorbs the ~40us first-collective cost.
"""

import sys

sys.path.insert(0, "/opt/trn_rl_repo")

import numpy as np

L, B, S, D, H, DK = 4, 2, 1024, 512, 8, 64
TOPK = 16
EPS = 1e-6
SCALE = 1.0 / np.sqrt(DK)
NT = S // 128  # token tiles
NDT = D // 128  # d-dim tiles

# AllReduce token segments: tiles [0,4), [4,6), [6,7), [7,8) — fired after
# qt 3/5/6/7; the small tail segments land quickly after attention ends.
SEGS = [(0, 2), (2, 4), (4, 6), (6, 7), (7, 8)]
SEG_FIRE = {1: 0, 3: 1, 5: 2, 6: 3, 7: 4}

_COMPILED = None


# --------------------------------------------------------------------------
# Custom DVE select ops with hand-written 2x/4x perf-mode uop programs.
# SELZ4X_ANT:  out = in0 * (in0 <  s0);  SELGE4X_ANT: out = in0 * (in0 >= s0)
# Single-tensor-stream ops (Src0 read on two input-mux lanes), so with
# perf_max=3 the engine runs 2x_1p / 2x_2p / 4x_2p for fp16 step-1 SBUF
# operands (4x measured bit-exact at ~1/3 the 1x time). The variant programs
# mirror the stock tensor_scalar perf rows (gen3 default table slots 16..19):
# two ALU stages per element (CMP then MULTIPLY), elements packed
# e0=SRC_0, e1=SRC_0_HI, e2=SRC_1, e3=SRC_1_HI, results repacked via delay
# lanes to WR0_LO/WR0_HI/WR1_LO/WR1_HI.
# --------------------------------------------------------------------------
def _register_fast_selects():
    from concourse.dve_ops import OPS, DveOp, _COMPILE_CACHE
    import concourse.dve_ops as dops
    from concourse.dve_spec import Spec, Src0, C0, lower
    from concourse.dve_uop import (
        DISABLE, ENABLE, AluInp, AluOp, DelayInp, DveOpSpec, InpSel,
        OutPath, OutSel, Trigger, UopConfig, UopDpConfig,
    )

    def _mk_uop(inp, outs, blocks):
        u = UopConfig()
        for lane, sel in inp:
            u.enable_input(sel, lane)
        for path, sel in outs:
            u.enable_output(sel, path)
        u.require_inp0 = ENABLE
        u.require_inp1 = ENABLE if any(
            s in (InpSel.SRC_1, InpSel.SRC_1_HI) for _, s in inp) else DISABLE
        u.trigger = (Trigger.SRC_TENSOR_DONE, Trigger.NONE, Trigger.NONE)
        u.datapath_config = blocks
        return u

    def _blk(op=None, m0=None, m1=None, passthru=(), cap=None):
        b = UopDpConfig()
        if op is not None:
            b.enable_alu(op, m0, m1)
        b.pass_through_delay(*passthru)
        if cap is not None:
            b.enable_delay_from_src(cap[1], cap[0])
        return b

    _D = [AluInp.PREV_DELAY_0, AluInp.PREV_DELAY_1, AluInp.PREV_DELAY_2,
          AluInp.PREV_DELAY_3, AluInp.PREV_DELAY_4, AluInp.PREV_DELAY_5]
    _PREV = AluInp.PREV_ALU_OUT
    _CAP = DelayInp.PREV_ALU_OUT

    def _variant_4x(cmp_op):
        blocks = [
            _blk(cmp_op, _PREV, _D[0], passthru=(0, 1, 2, 3, 4)),
            _blk(AluOp.MULTIPLY, _PREV, _D[1], passthru=(0, 2, 3, 4)),
            _blk(cmp_op, _D[2], _D[0], passthru=(0, 2, 3, 4), cap=(1, _CAP)),
            _blk(AluOp.MULTIPLY, _PREV, _D[2], passthru=(0, 1, 3, 4)),
            _blk(cmp_op, _D[3], _D[0], passthru=(0, 1, 3, 4), cap=(2, _CAP)),
            _blk(AluOp.MULTIPLY, _PREV, _D[3], passthru=(0, 1, 2, 4)),
            _blk(cmp_op, _D[4], _D[0], passthru=(1, 2, 4), cap=(3, _CAP)),
            _blk(AluOp.MULTIPLY, _PREV, _D[4], passthru=(1, 2, 3)),
        ]
        return _mk_uop(
            inp=[(0, InpSel.SRC_0), (1, InpSel.CONST_0), (2, InpSel.SRC_0),
                 (3, InpSel.SRC_0_HI), (4, InpSel.SRC_1),
                 (5, InpSel.SRC_1_HI)],
            outs=[(OutPath.WR0_LO, OutSel.DELAY_1),
                  (OutPath.WR0_HI, OutSel.DELAY_2),
                  (OutPath.WR1_LO, OutSel.DELAY_3),
                  (OutPath.WR1_HI, OutSel.ALU_OUT)],
            blocks=blocks,
        )

    def _variant_2x(cmp_op, second_src):
        bypass = []
        for _ in range(4):
            b = UopDpConfig().pass_through_alu()
            b.pass_through_delay(1)
            bypass.append(b)
        blocks = [
            _blk(cmp_op, _PREV, _D[0], passthru=(0, 1, 2)),
            _blk(AluOp.MULTIPLY, _PREV, _D[1], passthru=(0, 2)),
            _blk(cmp_op, _D[2], _D[0], passthru=(2,), cap=(1, _CAP)),
            _blk(AluOp.MULTIPLY, _PREV, _D[2], passthru=(1,)),
        ] + bypass
        hi_path = (OutPath.WR0_HI if second_src is InpSel.SRC_0_HI
                   else OutPath.WR1_LO)
        return _mk_uop(
            inp=[(0, InpSel.SRC_0), (1, InpSel.CONST_0), (2, InpSel.SRC_0),
                 (3, second_src)],
            outs=[(OutPath.WR0_LO, OutSel.DELAY_1), (hi_path, OutSel.ALU_OUT)],
            blocks=blocks,
        )

    _CMP_FOR = {"SELZ4X_ANT": AluOp.IS_LT, "SELGE4X_ANT": AluOp.IS_GE}

    class PerfDveOp(DveOp):
        def compile(self, ver):
            key = (self.name, ver)
            if (r := _COMPILE_CACHE.get(key)) is not None:
                return r
            cmp_op = _CMP_FOR[self.name]
            kw = {}
            if ver == "v3":
                kw = dict(
                    uops_2x=[_variant_2x(cmp_op, InpSel.SRC_0_HI)],
                    uops_2x_2p=[_variant_2x(cmp_op, InpSel.SRC_1)],
                    uops_4x=[_variant_4x(cmp_op)],
                    perf_max=3,
                )
            result = DveOpSpec(
                name=self.name,
                opcode=dops.get_dve_sub_opcode(self.name),
                uops=lower(self.spec, ver=ver),
                rd1_en=False,
                **kw,
            )
            result.validate(ver)
            _COMPILE_CACHE[key] = result
            return result

    specs = {
        "SELZ4X_ANT": Spec(
            body=(Src0 < C0) * Src0,
            reference=lambda in0, in1, s0, s1, imm2: np.where(
                in0 < s0, in0, 0.0),
        ),
        "SELGE4X_ANT": Spec(
            body=(Src0 >= C0) * Src0,
            reference=lambda in0, in1, s0, s1, imm2: np.where(
                in0 >= s0, in0, 0.0),
        ),
    }
    out = {}
    for name, spec in specs.items():
        existing = next((op for op in OPS if op.name == name), None)
        if existing is not None:
            out[name] = existing
            continue
        op = PerfDveOp(name, spec, subdim=False, uops_sha={})
        OPS.append(op)
        dops._SUB_OPCODE_FOR_NAME[op.name] = (
            dops._CUSTOM_DVE_ROW_BASE + len(OPS) - 1
        )
        out[name] = op
    return out


def _fast_select(nc, op, out, in0, s0):
    """Emit the custom select with perf modes enabled (perf_max must be set
    at construction; add_instruction clones the instruction)."""
    import concourse.bass_isa as bass_isa
    import concourse.mybir as mybir
    import concourse.dve_ops as dops

    eng = nc.vector
    if op.name not in eng.bass.m.ant_custom_dve_ops:
        eng.bass.m.ant_custom_dve_ops = sorted(
            {*eng.bass.m.ant_custom_dve_ops, op.name}
        )
    op.compile("v3")
    shape = bass_isa.CustomDveShape.TTSS
    isa_opcode = eng.bass.isa.Opcode[
        f"NEURON_ISA_TPB_OPCODE_CUSTOM_DVE_ANT_{shape.slot()}"
    ].value
    zero = mybir.ImmediateValue(dtype=mybir.dt.float32, value=0.0)
    ins = [
        eng.lower_ap(in0, for_isa=True, opt=True),
        eng.lower_ap(s0, for_isa=True),
        zero,
    ]
    outs = [eng.lower_ap(out, for_isa=True, opt=True)]
    return eng.add_instruction(
        bass_isa.InstCustomDveAnt(
            name=eng.bass.get_next_instruction_name(),
            op_name=op.name,
            rd1_en=False,
            subdim=0,
            imm2=0.0,
            shape=shape,
            row=dops.get_dve_sub_opcode(op.name),
            isa_opcode=isa_opcode,
            perf_max=3,
            ins=ins,
            outs=outs,
        )
    )


def _build(sim=False):
    import concourse.bacc as bacc
    import concourse.mybir as mybir
    import concourse.tile as tile
    from concourse import masks

    f32 = mybir.dt.float32
    f32r = mybir.dt.float32r
    f16 = mybir.dt.float16
    bf16 = mybir.dt.bfloat16
    AL = mybir.AluOpType
    AF = mybir.ActivationFunctionType

    sel_ops = _register_fast_selects()
    SELZ = sel_ops["SELZ4X_ANT"]
    SELGE = sel_ops["SELGE4X_ANT"]

    nc = bacc.Bacc(
        "TRN2", target_bir_lowering=False, debug=False,
        num_devices=(1 if sim else 8),
    )

    # host-packed partition-major inputs (all contiguous DMAs)
    x_d = nc.dram_tensor("x", (128, NT * D), f32, kind="ExternalInput")
    xr_d = nc.dram_tensor("xr", (128, NT * D), f32, kind="ExternalInput")
    wq_d = nc.dram_tensor("wq", (128, L * NDT * 128), f32r, kind="ExternalInput")
    wk_d = nc.dram_tensor("wk", (128, L * NDT * 128), f32r, kind="ExternalInput")
    wv_d = nc.dram_tensor("wv", (128, L * NDT * 128), f32r, kind="ExternalInput")
    wo_d = nc.dram_tensor("wo", (128, L * D), f32r, kind="ExternalInput")
    bq_d = nc.dram_tensor("bq", (L, 128), f32r, kind="ExternalInput")
    onesr_d = nc.dram_tensor("onesr", (1, 512), f32r, kind="ExternalInput")
    # rows: [0:L] beta, [L:2L] C[l] (= gamma[l] + B[l+1]; C[L-1] = gamma[L-1])
    rows_d = nc.dram_tensor("rows", (2 * L, D), f32, kind="ExternalInput")
    out_d = nc.dram_tensor("out", (S, D), f32, kind="ExternalOutput")

    cc_in = [
        [nc.dram_tensor(f"cc_in{l}_{s}", ((b - a) * 128, D), bf16, kind="Internal")
         for s, (a, b) in enumerate(SEGS)] for l in range(L)
    ]
    cc_out = [
        [nc.dram_tensor(f"cc_out{l}_{s}", ((b - a) * 128, D), bf16, kind="Internal")
         for s, (a, b) in enumerate(SEGS)] for l in range(L)
    ]
    ccw_in = nc.dram_tensor("ccw_in", (128, 64), bf16, kind="Internal")
    ccw_out = nc.dram_tensor("ccw_out", (128, 64), bf16, kind="Internal")
    GROUPS = [[0, 1, 2, 3], [4, 5, 6, 7]]

    with tile.TileContext(nc) as tc:
        with (
            tc.tile_pool(name="w", bufs=1) as wp,
            tc.tile_pool(name="state", bufs=1) as st,
            tc.tile_pool(name="sb", bufs=3) as sb,
            tc.tile_pool(name="sm", bufs=3) as sm,
            tc.tile_pool(name="ps_w", bufs=2, space="PSUM") as ps_w,
            tc.tile_pool(name="ps_s", bufs=2, space="PSUM") as ps_s,
            tc.tile_pool(name="ps_p", bufs=2, space="PSUM") as ps_p,
            tc.tile_pool(name="ps_o", bufs=2, space="PSUM") as ps_o,
        ):
            ident = wp.tile([128, 128], f32, tag="ident", name="ident")
            masks.make_identity(nc, ident[:])
            ident16 = wp.tile([128, 128], f16, tag="ident16", name="ident16")
            nc.scalar.copy(ident16[:], ident[:])

            # ---- state ----
            x_sb = st.tile([128, NT, D], f32, tag="x", name="x_sb")
            r_sb = st.tile([128, NT, D], f32, tag="r", name="r_sb")
            n_sb = st.tile([128, NT, D], f32, tag="n", name="n_sb")
            hT_sb = st.tile([128, NDT, S], f32r, tag="hT", name="hT_sb")
            qT_sb = st.tile([128, S], f32r, tag="qT", name="qT_sb")
            kT_sb = st.tile([128, S], f32r, tag="kT", name="kT_sb")
            vT_sb = st.tile([128, S], f16, tag="vT", name="vT_sb")
            v_sb = st.tile([128, NT, 128], f16, tag="v", name="v_sb")

            # ---- weight tiles ----
            wq_sb = wp.tile([128, L, NDT, 128], f32r, tag="wq", name="wq_sb")
            wk_sb = wp.tile([128, L, NDT, 128], f32r, tag="wk", name="wk_sb")
            wv_sb = wp.tile([128, L, NDT, 128], f32r, tag="wv", name="wv_sb")
            wo_sb = wp.tile([128, L, D], f32r, tag="wo", name="wo_sb")
            bq_sb = [
                wp.tile([1, 128], f32r, name=f"bqs{l}", tag=f"bq{l}")
                for l in range(L)
            ]
            ones_row = wp.tile([1, 512], f32r, tag="ones_row", name="ones_row")
            rows_row = wp.tile([1, 2 * L, D], f32, tag="rows_row", name="rows_row")
            rows_bc = wp.tile([128, 2 * L, D], f32, tag="rows_bc", name="rows_bc")

            # ---- DMA: first-needed first; spread across engine queues ----
            LW = NDT * 128  # per-layer columns in packed w tensors
            x_r = x_d[:].rearrange("p (c d) -> p c d", c=NT)
            nc.sync.dma_start(
                wq_sb[:, 0], wq_d[:, 0:LW].rearrange("p (kc m) -> p kc m", kc=NDT)
            )
            nc.scalar.dma_start(
                wk_sb[:, 0], wk_d[:, 0:LW].rearrange("p (kc m) -> p kc m", kc=NDT)
            )
            nc.sync.dma_start(x_sb[:, 0:3], x_r[:, 0:3])
            nc.scalar.dma_start(x_sb[:, 3:6], x_r[:, 3:6])
            nc.sync.dma_start(
                r_sb[:], xr_d[:].rearrange("p (c d) -> p c d", c=NT)
            )
            nc.scalar.dma_start(ones_row[:], onesr_d[:])
            for l in range(L):
                nc.scalar.dma_start(bq_sb[l][:], bq_d[l : l + 1, :])
            nc.gpsimd.dma_start(
                wv_sb[:, 0], wv_d[:, 0:LW].rearrange("p (kc m) -> p kc m", kc=NDT)
            )
            nc.gpsimd.dma_start(x_sb[:, 6:8], x_r[:, 6:8])
            nc.gpsimd.dma_start(
                rows_row[:], rows_d[:].rearrange("(o r) d -> o r d", o=1)
            )
            for r in range(2 * L):
                nc.gpsimd.partition_broadcast(rows_bc[:, r], rows_row[:, r])
            nc.scalar.dma_start(wo_sb[:, 0], wo_d[:, 0:D])
            for l in range(1, L):
                for w_sb, w_d in ((wq_sb, wq_d), (wk_sb, wk_d), (wv_sb, wv_d)):
                    nc.gpsimd.dma_start(
                        w_sb[:, l],
                        w_d[:, l * LW : (l + 1) * LW].rearrange(
                            "p (kc m) -> p kc m", kc=NDT
                        ),
                    )
                nc.gpsimd.dma_start(wo_sb[:, l], wo_d[:, l * D : (l + 1) * D])
            # warmup collective: absorbs first-CC latency; emitted after the
            # gpsimd DMA/broadcast stream so it doesn't block those.
            if not sim:
                nc.gpsimd.collective_compute(
                    "AllReduce", mybir.AluOpType.add,
                    replica_groups=GROUPS,
                    ins=[ccw_in[:]], outs=[ccw_out[:]],
                )

            # ---------------- helpers ----------------
            def prep_half(l, src_sb, half):
                """Transposes of token tiles [half*4, half*4+4) + QKV nh=half
                (+ v transposes of this half's key tiles). Transposes are
                emitted per token tile so tile t's transpose overlaps the LN
                of tile t+1."""
                cs = slice(half * 512, (half + 1) * 512)
                for c in range(4):
                    ct = half * 4 + c
                    tp = ps_w.tile([128, 512], f32, tag="work",
                                   name=f"tp{l}_{ct}")
                    for dt in range(NDT):
                        nc.tensor.transpose(
                            tp[:, dt * 128 : (dt + 1) * 128],
                            src_sb[:, ct, dt * 128 : (dt + 1) * 128],
                            ident[:],
                        )
                    nc.scalar.copy(
                        hT_sb[:, :, ct * 128 : (ct + 1) * 128],
                        tp[:].rearrange("p (dt m) -> p dt m", dt=NDT),
                    )
                qp = ps_w.tile([128, 512], f32, tag="work", name=f"qp{l}_{half}")
                for dt in range(NDT):
                    nc.tensor.matmul(
                        qp[:], wq_sb[:, l, dt], hT_sb[:, dt, cs],
                        start=(dt == 0), stop=False,
                    )
                nc.tensor.matmul(
                    qp[:], bq_sb[l][:], ones_row[:], start=False, stop=True
                )
                nc.scalar.copy(qT_sb[:, cs], qp[:])
                kp = ps_w.tile([128, 512], f32, tag="work", name=f"kp{l}_{half}")
                for dt in range(NDT):
                    nc.tensor.matmul(
                        kp[:], wk_sb[:, l, dt], hT_sb[:, dt, cs],
                        start=(dt == 0), stop=(dt == NDT - 1),
                    )
                nc.scalar.copy(kT_sb[:, cs], kp[:])
                vp = ps_w.tile([128, 512], f32, tag="work", name=f"vp{l}_{half}")
                for dt in range(NDT):
                    nc.tensor.matmul(
                        vp[:], wv_sb[:, l, dt], hT_sb[:, dt, cs],
                        start=(dt == 0), stop=(dt == NDT - 1),
                    )
                nc.scalar.copy(vT_sb[:, cs], vp[:])  # cast fp16
                vtp = ps_p.tile([128, 4, 128], f16, tag="pt",
                                name=f"vtp{l}_{half}")
                for c in range(4):
                    kc = half * 4 + c
                    nc.tensor.transpose(
                        vtp[:, c], vT_sb[:, kc * 128 : (kc + 1) * 128], ident16[:]
                    )
                nc.vector.tensor_copy(
                    v_sb[:, half * 4 : half * 4 + 4], vtp[:]
                )

            def attention(l):
                for qt in range(NT):
                    oT_ps = ps_o.tile([128, 128], f32, tag="oT",
                                      name=f"oT{l}_{qt}")
                    z2 = sm.tile([128, 2], f32, tag="z2", name=f"z2{l}_{qt}")
                    iz2 = sm.tile([128, 2], f32, tag="iz2", name=f"iz2{l}_{qt}")
                    for h in range(2):
                        hs = slice(h * 64, (h + 1) * 64)
                        e = sb.tile([128, S], f16, tag="e", name=f"e{l}_{qt}_{h}")
                        for nh in range(2):
                            cs = slice(nh * 512, (nh + 1) * 512)
                            s_ps = ps_s.tile(
                                [128, 512], f32, tag="s",
                                name=f"sps{l}_{qt}_{h}_{nh}",
                            )
                            nc.tensor.matmul(
                                s_ps[:],
                                qT_sb[hs, qt * 128 : (qt + 1) * 128],
                                kT_sb[hs, cs],
                                start=True, stop=True,
                            )
                            nc.scalar.activation(
                                e[:, cs], s_ps[:], AF.Exp, scale=float(SCALE)
                            )
                        # top-16: max8 -> zero top-8 -> max8; Z = sum of the
                        # 16 maxima via one scalar accum over the m16 pair.
                        m16 = sm.tile([128, 16], f32, tag="m16",
                                      name=f"m16{l}_{qt}_{h}")
                        nc.vector.max(m16[:, 0:8], e[:])
                        e2 = sb.tile([128, S], f16, tag="e2",
                                     name=f"e2{l}_{qt}_{h}")
                        _fast_select(nc, SELZ, e2[:], e[:], m16[:, 7:8])
                        nc.vector.max(m16[:, 8:16], e2[:])
                        p = sb.tile([128, S], f16, tag="p", name=f"p{l}_{qt}_{h}")
                        _fast_select(nc, SELGE, p[:], e[:], m16[:, 15:16])
                        dm = sm.tile([128, 16], f32, tag="dm",
                                     name=f"dm{l}_{qt}_{h}")
                        nc.scalar.activation(
                            dm[:], m16[:], AF.Copy,
                            accum_out=z2[:, h : h + 1],
                        )
                        pT_ps = ps_p.tile([128, NT, 128], f16, tag="pt",
                                          name=f"pT{l}_{qt}_{h}")
                        for kc in range(NT):
                            nc.tensor.transpose(
                                pT_ps[:, kc], p[:, kc * 128 : (kc + 1) * 128],
                                ident16[:],
                            )
                        pT = sb.tile([128, NT, 128], f16, tag="pT",
                                     name=f"pTs{l}_{qt}_{h}")
                        if (qt + h) % 2 == 0:
                            nc.vector.tensor_copy(pT[:], pT_ps[:])
                        else:
                            nc.scalar.copy(pT[:], pT_ps[:])
                        for kc in range(NT):
                            nc.tensor.matmul(
                                oT_ps[hs, :],
                                v_sb[:, kc, hs],
                                pT[:, kc],
                                start=(kc == 0), stop=(kc == NT - 1),
                            )
                    nc.vector.reciprocal(iz2[:], z2[:])
                    oT_sb = sb.tile([128, 128], f32r, tag="oTsb",
                                    name=f"oTsb{l}_{qt}")
                    nc.scalar.copy(oT_sb[:], oT_ps[:])
                    ys = []
                    for h in range(2):
                        hs = slice(h * 64, (h + 1) * 64)
                        y_ps = ps_w.tile([128, 512], f32, tag="work",
                                         name=f"yps{l}_{qt}_{h}")
                        nc.tensor.matmul(
                            y_ps[:], oT_sb[hs, :], wo_sb[hs, l],
                            start=True, stop=True,
                        )
                        yh = sb.tile([128, 512], bf16, tag=f"ysc{h}",
                                     name=f"ysc{l}_{qt}_{h}")
                        nc.scalar.activation(
                            yh[:], y_ps[:], AF.Copy, scale=iz2[:, h : h + 1],
                        )
                        ys.append(yh)
                    y_sb = sb.tile([128, D], bf16, tag="y_sb",
                                   name=f"ysb{l}_{qt}")
                    nc.vector.tensor_add(y_sb[:], ys[0][:], ys[1][:])
                    seg = next(s for s, (a, b) in enumerate(SEGS)
                               if a <= qt < b)
                    row0 = (qt - SEGS[seg][0]) * 128
                    nc.sync.dma_start(
                        cc_in[l][seg][row0 : row0 + 128, :], y_sb[:]
                    )
                    if qt in SEG_FIRE:
                        seg = SEG_FIRE[qt]
                        if sim:
                            nt_seg = SEGS[seg][1] - SEGS[seg][0]
                            for rr in range(nt_seg):
                                cpt = sb.tile([128, D], bf16, tag="cp",
                                              name=f"cp{l}_{seg}_{rr}")
                                nc.sync.dma_start(
                                    cpt[:],
                                    cc_in[l][seg][rr * 128 : (rr + 1) * 128, :],
                                )
                                nc.sync.dma_start(
                                    cc_out[l][seg][rr * 128 : (rr + 1) * 128, :],
                                    cpt[:],
                                )
                        else:
                            nc.gpsimd.collective_compute(
                                "AllReduce",
                                mybir.AluOpType.add,
                                replica_groups=GROUPS,
                                ins=[cc_in[l][seg][:]],
                                outs=[cc_out[l][seg][:]],
                            )

            def ln_tile(l, t):
                """Residual + LayerNorm -> n_sb[t] (normalized, no affine)."""
                seg = next(s for s, (a, b) in enumerate(SEGS) if a <= t < b)
                row0 = (t - SEGS[seg][0]) * 128
                yt = sb.tile([128, D], bf16, tag="yt", name=f"yt{l}_{t}")
                nc.sync.dma_start(yt[:], cc_out[l][seg][row0 : row0 + 128, :])
                y1 = sb.tile([128, D], f32, tag="y1", name=f"y1{l}_{t}")
                nc.vector.tensor_add(y1[:], yt[:], r_sb[:, t, :])
                stats = sm.tile([128, 6], f32, tag="stats", name=f"st{l}_{t}")
                nc.vector.bn_stats(stats[:], y1[:])
                mv = sm.tile([128, 2], f32, tag="mv", name=f"mv{l}_{t}")
                nc.vector.bn_aggr(mv[:], stats[:])
                # rstd ~ 1/(std+eps) ~ 1/sqrt(var * D/(D-1))  (eps=1e-6 negligible)
                std = sm.tile([128, 1], f32, tag="std", name=f"sd{l}_{t}")
                nc.scalar.activation(
                    std[:], mv[:, 1:2], AF.Sqrt, scale=float(D / (D - 1))
                )
                rstd = sm.tile([128, 1], f32, tag="rstd", name=f"rs{l}_{t}")
                nc.vector.reciprocal(rstd[:], std[:])
                nc.vector.tensor_scalar(
                    n_sb[:, t, :], y1[:], mv[:, 0:1], rstd[:],
                    op0=AL.subtract, op1=AL.mult,
                )

            def r_tile(l, t):
                """r = beta_l * n + C_l (GpSimd, off critical path).
                For the last layer r IS the output; split across engines
                to drain the tail faster."""
                if l == L - 1 and t % 2 == 0:
                    nc.vector.tensor_mul(r_sb[:, t, :], n_sb[:, t, :],
                                         rows_bc[:, l])
                    nc.vector.tensor_add(r_sb[:, t, :], r_sb[:, t, :],
                                         rows_bc[:, L + l])
                else:
                    nc.gpsimd.tensor_mul(r_sb[:, t, :], n_sb[:, t, :],
                                         rows_bc[:, l])
                    nc.gpsimd.tensor_add(r_sb[:, t, :], r_sb[:, t, :],
                                         rows_bc[:, L + l])
                if l == L - 1:
                    nc.sync.dma_start(
                        out_d[t * 128 : (t + 1) * 128, :], r_sb[:, t, :]
                    )

            # ---------------- main schedule ----------------
            prep_half(0, x_sb, 0)
            prep_half(0, x_sb, 1)
            for l in range(L):
                attention(l)
                for half in range(2):
                    for t in range(half * 4, half * 4 + 4):
                        ln_tile(l, t)
                    if l < L - 1:
                        prep_half(l + 1, n_sb, half)
                for t in range(NT):
                    r_tile(l, t)

    nc.compile()
    return nc


def _get_compiled():
    global _COMPILED
    if _COMPILED is None:
        _COMPILED = _build()
    return _COMPILED


def _host_prep(x, Wq, Wk, Wv, Wo, bq, bk, bv, bo, gamma, beta):
    """Build the 8 per-core input maps (all arrays partition-major packed)."""
    f64 = np.float64
    Wq, Wk, Wv, Wo = [a.astype(f64) for a in (Wq, Wk, Wv, Wo)]
    bq, bk, bv, bo, gamma, beta = [
        a.astype(f64) for a in (bq, bk, bv, bo, gamma, beta)
    ]
    # fold previous-layer LN affine into weights:
    # l=0 consumes raw x; l>=1 consumes n^{l-1}:
    #   h = beta_{l-1} * n + gamma_{l-1}
    #   h @ W = n @ (diag(beta)W) + gamma @ W
    Wq_e = np.stack([Wq[l] if l == 0 else beta[l - 1][:, None] * Wq[l]
                     for l in range(L)])
    Wk_e = np.stack([Wk[l] if l == 0 else beta[l - 1][:, None] * Wk[l]
                     for l in range(L)])
    Wv_e = np.stack([Wv[l] if l == 0 else beta[l - 1][:, None] * Wv[l]
                     for l in range(L)])
    bq_e = np.stack([bq[l] if l == 0 else bq[l] + gamma[l - 1] @ Wq[l]
                     for l in range(L)])
    # bk dropped: adds a per-query-row constant to scores (softmax/topk
    # invariant). bv folded into the bias row B below.
    bv_e = np.stack([bv[l] if l == 0 else bv[l] + gamma[l - 1] @ Wv[l]
                     for l in range(L)])
    Bl = np.stack([bv_e[l] @ Wo[l] + bo[l] for l in range(L)])  # [L, D]
    # residual carrier constants: y_{l+1} = o@Wo + B[l+1] + beta_l*n + gamma_l
    C = np.stack([gamma[l] + (Bl[l + 1] if l + 1 < L else 0.0)
                  for l in range(L)])
    rows = np.concatenate([beta, C], axis=0).astype(np.float32)  # [2L, D]

    in_maps = []
    for c in range(8):
        b, r = divmod(c, 4)
        cols = slice(128 * r, 128 * (r + 1))
        # partition-major packs
        xb = x[b].astype(f64)  # [S, D]
        x_pm = np.ascontiguousarray(
            xb.reshape(NT, 128, D).transpose(1, 0, 2).reshape(128, NT * D)
        ).astype(np.float32)
        xr_pm = np.ascontiguousarray(
            (xb + Bl[0]).reshape(NT, 128, D).transpose(1, 0, 2).reshape(
                128, NT * D)
        ).astype(np.float32)

        def pack_w(W_e):
            # [L, D, 128cols] -> [128p, L*NDT*128m]; p = contract row % 128
            wl = W_e[:, :, cols]  # [L, D, 128]
            return np.ascontiguousarray(
                wl.reshape(L, NDT, 128, 128).transpose(2, 0, 1, 3).reshape(
                    128, L * NDT * 128)
            ).astype(np.float32)

        wo_pm = np.ascontiguousarray(
            Wo[:, cols, :].transpose(1, 0, 2).reshape(128, L * D)
        ).astype(np.float32)
        in_maps.append(
            {
                "x": x_pm,
                "xr": xr_pm,
                "wq": pack_w(Wq_e),
                "wk": pack_w(Wk_e),
                "wv": pack_w(Wv_e),
                "wo": wo_pm,
                "bq": np.ascontiguousarray(bq_e[:, cols]).astype(np.float32),
                "onesr": np.ones((1, 512), np.float32),
                "rows": rows,
            }
        )
    return in_maps


def _numpy_fallback(x, mask, Wq, Wk, Wv, Wo, bq, bk, bv, bo, gamma, beta):
    m = np.asarray(mask)[:, None, :, :]
    h = np.asarray(x, dtype=np.float64)
    for l in range(L):
        q = (h @ Wq[l] + bq[l]).reshape(B, S, H, DK).transpose(0, 2, 1, 3)
        k = (h @ Wk[l] + bk[l]).reshape(B, S, H, DK).transpose(0, 2, 1, 3)
        v = (h @ Wv[l] + bv[l]).reshape(B, S, H, DK).transpose(0, 2, 1, 3)
        s = np.einsum("bhqd,bhkd->bhqk", q, k) * SCALE
        kth = np.sort(s, axis=-1)[..., -TOPK][..., None]
        keep = (s >= kth) & m
        sm = np.where(keep, s, -1e9)
        sm = sm - sm.max(-1, keepdims=True)
        p = np.exp(sm)
        p /= p.sum(-1, keepdims=True)
        o = np.einsum("bhqk,bhkd->bhqd", p, v)
        o = o.transpose(0, 2, 1, 3).reshape(B, S, D) @ Wo[l] + bo[l]
        y = h + o
        mean = y.mean(-1, keepdims=True)
        std = y.std(-1, ddof=1, keepdims=True)
        h = beta[l] * (y - mean) / (std + EPS) + gamma[l]
    return h.astype(np.float32)


def kernel(x, mask, Wq, Wk, Wv, Wo, bq, bk, bv, bo, gamma, beta):
    x = np.asarray(x, dtype=np.float32)
    mask_np = np.asarray(mask)
    args = [
        np.asarray(a, dtype=np.float32)
        for a in (Wq, Wk, Wv, Wo, bq, bk, bv, bo, gamma, beta)
    ]
    if not mask_np.all():
        return _numpy_fallback(x, mask_np, *args)

    from concourse import bass_utils

    nc = _get_compiled()
    in_maps = _host_prep(x, *args)
    res = bass_utils.run_bass_kernel_spmd(nc, in_maps, core_ids=list(range(8)))
    out = np.stack([res.results[0]["out"], res.results[4]["out"]])
    return out.astype(np.float32)
